# revision 34
# baseline (speedup 1.0000x reference)
"""Trainium2 Bass kernel for HGATLinkConv (GNN message passing).

Strategy (8 NeuronCores, SPMD):
  - dst nodes are partitioned contiguously across cores (1250/core); each
    core's edges are those with dst in its range (host-side index prep).
  - Each core computes h = relu((feat @ W) * cj) for ALL nodes (sources can be
    anywhere) via PE matmuls, stores the [N,128] f32 table to DRAM scratch.
  - segment_max: local dst nodes are sorted by in-degree (host).  Round k
    gathers the k-th neighbor's h-row of every node with degree > k (a dense
    prefix of the sorted order), via gpsimd.dma_gather (one 512B row per
    edge), and DVE tensor_max-accumulates into a [128, npos] accumulator
    where position i lives at partition i%128, block i//128 (exactly the
    dma_gather output layout).  Messages are >= 0 and the reference clamps
    the result at 0, so a zero accumulator init + padding with a guaranteed
    zero row is exact.
  - Attention gate (wk = feat @ Wk, per-head L2-normalized q, softmax over
    features) is computed for local nodes only, on ACT/DVE, overlapping the
    gather phase.  Final out = rst * attn.
  - Host un-permutes rows and assembles the full [10000, 128] output.
"""

import numpy as np
from contextlib import ExitStack

import concourse.bacc as bacc
import concourse.bass as bass
import concourse.mybir as mybir
import concourse.tile as tile
from concourse.tile_rust import add_dep_helper

F32 = mybir.dt.float32
BF16 = mybir.dt.bfloat16
I16 = mybir.dt.int16
AFT = mybir.ActivationFunctionType
ALU = mybir.AluOpType

# problem constants (hardcoded; kernel.py must be self-contained)
N = 10000
E = 640000
IN_F = 256
OUT_F = 128
HEADS = 8
D_K = 16
TAU = 0.25
NCORES = 8


def _ceil_to(x, m):
    return (x + m - 1) // m * m


def plan(src, dst, n, nloc, ncores, chunk_blocks):
    """Host-side index planning.  Returns per-core permutations, device-layout
    gather index arrays, the global (SPMD-uniform) per-chunk DVE segment
    schedule, and the total block count TB."""
    src = np.asarray(src).astype(np.int64)
    dst = np.asarray(dst).astype(np.int64)
    core_of = dst // nloc
    percore = []
    for c in range(ncores):
        m = core_of == c
        s_c = src[m]
        d_c = dst[m] - c * nloc
        deg = np.bincount(d_c, minlength=nloc)
        perm = np.argsort(-deg, kind="stable")
        sdeg = deg[perm]
        order = np.argsort(d_c, kind="stable")
        s_sorted = s_c[order]
        offs = np.concatenate([[0], np.cumsum(deg)])
        percore.append((perm, sdeg, s_sorted, offs))
    maxdeg = int(max(int(p[1][0]) if len(p[1]) else 0 for p in percore))
    ks = np.arange(maxdeg)
    # n_k per core = number of local nodes with degree > k
    nks = np.stack([(p[1][None, :] > ks[:, None]).sum(1) for p in percore])
    bk = np.maximum(1, -(-nks.max(0) // 128))  # blocks per round, global
    tb0 = int(bk.sum())
    tb = _ceil_to(max(tb0, chunk_blocks), chunk_blocks)
    nchunks = tb // chunk_blocks
    starts = np.concatenate([[0], np.cumsum(bk)])
    segments = [[] for _ in range(nchunks)]
    for k in range(maxdeg):
        gb = int(starts[k])
        b0 = 0
        while b0 < bk[k]:
            chunk, off = divmod(gb, chunk_blocks)
            take = int(min(bk[k] - b0, chunk_blocks - off))
            segments[chunk].append((off, b0, take))
            gb += take
            b0 += take
    zrow = n  # first padded (guaranteed-zero) row of the h table
    idx_arrs = []
    for ci_, (perm, sdeg, s_sorted, offs) in enumerate(percore):
        flat = np.full(tb * 128, zrow, np.int64)
        for k in range(maxdeg):
            nk = int(nks[ci_][k])
            if nk == 0:
                continue
            tgt = offs[perm[:nk]] + k
            flat[int(starts[k]) * 128: int(starts[k]) * 128 + nk] = s_sorted[tgt]
        wrapped = flat.astype(np.int16).reshape(-1, 16).T  # [16, tb*8]
        idx_arrs.append(np.ascontiguousarray(np.tile(wrapped, (8, 1))))
    perms = [p[0] for p in percore]
    return perms, idx_arrs, segments, tb


def build(n, in_f, out_f, heads, d_k, tau, nloc, tb, segments, chunk_blocks):
    """Build the SPMD Bass program (same structure for every core)."""
    npos = _ceil_to(nloc, 128)
    npad = _ceil_to(n + 1, 1024)
    nchunks = tb // chunk_blocks
    idx_cols = tb * 8
    nmt_l = npos // 128

    nc = bacc.Bacc("TRN2", target_bir_lowering=False, debug=False,
                   num_swdge_queues=4)
    featT_g = nc.dram_tensor("featT_g", [in_f, npad], BF16, kind="ExternalInput")
    featT_l = nc.dram_tensor("featT_l", [in_f, npos], F32, kind="ExternalInput")
    w_d = nc.dram_tensor("w", [in_f, out_f], BF16, kind="ExternalInput")
    wk_d = nc.dram_tensor("wk", [in_f, out_f], F32, kind="ExternalInput")
    cj_d = nc.dram_tensor("cj_sb", [128, npad // 128], F32, kind="ExternalInput")
    ci_d = nc.dram_tensor("ci_sb", [128, nmt_l], F32, kind="ExternalInput")
    idx_d = nc.dram_tensor("idxs", [128, idx_cols], I16, kind="ExternalInput")
    h_d = nc.dram_tensor("h_scratch", [npad, out_f], BF16)
    out_d = nc.dram_tensor("out", [128, npos], F32, kind="ExternalOutput")

    with tile.TileContext(nc) as tc, ExitStack() as ctx:
        const = ctx.enter_context(tc.tile_pool(name="const", bufs=1))
        w0t = const.tile([128, out_f], BF16, tag="w0")
        w1t = const.tile([128, out_f], BF16, tag="w1")
        wk0t = const.tile([128, out_f], F32, tag="wk0")
        wk1t = const.tile([128, out_f], F32, tag="wk1")
        nc.sync.dma_start(w0t[:], w_d[0:128, :])
        nc.sync.dma_start(w1t[:], w_d[128:256, :])
        nc.sync.dma_start(wk0t[:], wk_d[0:128, :])
        nc.sync.dma_start(wk1t[:], wk_d[128:256, :])
        cit = const.tile([128, nmt_l], F32, tag="ci")
        nc.sync.dma_start(cit[:], ci_d[:, :])
        idxt = const.tile([128, idx_cols], I16, tag="idx")
        nc.sync.dma_start(idxt[:], idx_d[:, :])
        fl0 = const.tile([128, npos], F32, tag="fl0")
        fl1 = const.tile([128, npos], F32, tag="fl1")
        nc.sync.dma_start(fl0[:], featT_l[0:128, :])
        nc.sync.dma_start(fl1[:], featT_l[128:256, :])
        acc = const.tile([128, npos], BF16, tag="acc")
        attn = const.tile([128, npos], F32, tag="attn")
        nc.vector.memset(acc[:], 0.0)

        fpool = ctx.enter_context(tc.tile_pool(name="fpool", bufs=3))
        hpool = ctx.enter_context(tc.tile_pool(name="hpool", bufs=3))
        pspool = ctx.enter_context(
            tc.tile_pool(name="ps", bufs=6, space=bass.MemorySpace.PSUM))
        pstpool = ctx.enter_context(
            tc.tile_pool(name="pst", bufs=2, space=bass.MemorySpace.PSUM))
        apool = ctx.enter_context(tc.tile_pool(name="apool", bufs=2))
        gpool = ctx.enter_context(
            tc.tile_pool(name="gpool", bufs=6 if chunk_blocks <= 16 else 2))

        # ---- phase A: h = relu(feat_cj @ W) for all nodes -> DRAM ----
        # cj is folded into featT_g on the host.  Weight-stationary matmul
        # yields feature-major hT [out_f, nodes]; the XBAR dma-transpose
        # converts each [128, 512] tile to node-major [128, 4, 128] which is
        # stored contiguously as 512 rows of h_d.
        h_stores = []
        ch_cols = min(1024, npad)
        for c0 in range(0, npad, ch_cols):
            f0 = fpool.tile([128, ch_cols], BF16, tag="f0")
            f1 = fpool.tile([128, ch_cols], BF16, tag="f1")
            nc.sync.dma_start(f0[:], featT_g[0:128, c0:c0 + ch_cols])
            nc.sync.dma_start(f1[:], featT_g[128:256, c0:c0 + ch_cols])
            for t in range(ch_cols // 512):
                n0 = c0 + t * 512
                ps = pspool.tile([128, 512], F32, tag="ps")
                nc.tensor.matmul(ps[:], w0t[:], f0[:, t * 512:(t + 1) * 512],
                                 start=True, stop=False)
                nc.tensor.matmul(ps[:], w1t[:], f1[:, t * 512:(t + 1) * 512],
                                 start=False, stop=True)
                hTt = hpool.tile([128, 512], BF16, tag="hT")
                nc.scalar.activation(hTt[:], ps[:], AFT.Relu)
                hNt = hpool.tile([128, 512], BF16, tag="hN")
                hN3 = hNt[:].rearrange("p (m f) -> p m f", f=out_f)
                nc.sync.dma_start_transpose(hN3, hTt[:])
                st = nc.sync.dma_start(
                    h_d[n0:n0 + 512, :].rearrange("(m p) f -> p m f", p=128),
                    hN3)
                h_stores.append(st)

        # ---- phase C: attention gate for local nodes (overlaps B) ----
        qs = const.tile([128, npos], F32, tag="qs")       # q*q staging
        es = const.tile([128, npos], F32, tag="es")       # exp staging
        hs_all = const.tile([128, heads * nmt_l], F32, tag="hs_all")
        ssum_all = const.tile([128, nmt_l], F32, tag="ssum_all")
        inv_all = const.tile([128, heads * nmt_l], F32, tag="inv_all")
        sinv_all = const.tile([128, nmt_l], F32, tag="sinv_all")
        for t in range(nmt_l):
            ps = pspool.tile([128, out_f], F32, tag="ps")
            nc.tensor.matmul(ps[:], fl0[:, t * 128:(t + 1) * 128], wk0t[:],
                             start=True, stop=False)
            nc.tensor.matmul(ps[:], fl1[:, t * 128:(t + 1) * 128], wk1t[:],
                             start=False, stop=True)
            q = apool.tile([128, out_f], F32, tag="q")
            nc.scalar.activation(q[:], ps[:], AFT.Copy, scale=cit[:, t:t + 1])
            s = qs[:, t * 128:(t + 1) * 128]
            nc.vector.tensor_mul(s, q[:], q[:])
            s3 = s.rearrange("p (h d) -> p h d", d=d_k)
            nc.vector.reduce_sum(hs_all[:, t * heads:(t + 1) * heads], s3,
                                 axis=mybir.AxisListType.X)
        hsm = const.tile([128, heads * nmt_l], F32, tag="hsm")
        nc.vector.tensor_scalar_max(hsm[:], hs_all[:], 1e-24)
        nc.vector.reciprocal(inv_all[:], hsm[:])
        for t in range(nmt_l):
            s3 = qs[:, t * 128:(t + 1) * 128].rearrange(
                "p (h d) -> p h d", d=d_k)
            alpha = apool.tile([128, out_f], F32, tag="alpha")
            a3 = alpha[:].rearrange("p (h d) -> p h d", d=d_k)
            inv = inv_all[:, t * heads:(t + 1) * heads]
            nc.vector.tensor_tensor(a3, s3,
                                    inv.broadcast_to([128, heads, d_k]),
                                    op=ALU.mult)
            nc.scalar.activation(es[:, t * 128:(t + 1) * 128], alpha[:],
                                 AFT.Exp, scale=1.0 / tau,
                                 accum_out=ssum_all[:, t:t + 1])
        nc.vector.reciprocal(sinv_all[:], ssum_all[:])
        for t in range(nmt_l):
            nc.scalar.activation(attn[:, t * 128:(t + 1) * 128],
                                 es[:, t * 128:(t + 1) * 128],
                                 AFT.Copy, scale=sinv_all[:, t:t + 1])

        # ---- phase B: gather + segment-max (node-major) ----
        cb8 = chunk_blocks * 8
        n_idx = chunk_blocks * 128
        for chk in range(nchunks):
            g = gpool.tile([128, chunk_blocks * out_f], BF16, tag="g")
            g3 = g[:].rearrange("p (b e) -> p b e", e=out_f)
            gi = nc.gpsimd.dma_gather(
                g3, h_d[:, :], idxt[:, chk * cb8:(chk + 1) * cb8],
                n_idx, n_idx, out_f, elem_step=out_f,
                queue_num=chk % 4)
            for st in h_stores:
                add_dep_helper(gi.ins, st.ins, sync=True,
                               reason="gather reads full h table")
            for gb, ab, nb in segments[chk]:
                nc.vector.tensor_max(
                    acc[:, ab * 128:(ab + nb) * 128],
                    acc[:, ab * 128:(ab + nb) * 128],
                    g[:, gb * out_f:(gb + nb) * out_f])

        # ---- phase D: out = rst * attn ----
        o = const.tile([128, npos], F32, tag="o")
        nc.vector.tensor_mul(o[:], acc[:], attn[:])
        nc.sync.dma_start(out_d[:, :], o[:])

    nc.compile()
    return nc


def make_inputs(feat, ci, cj, weight, weight_k, perms, idx_arrs, n, nloc):
    import ml_dtypes
    bf16 = ml_dtypes.bfloat16
    feat = np.asarray(feat, np.float32)
    ci = np.asarray(ci, np.float32).reshape(-1)
    cj = np.asarray(cj, np.float32).reshape(-1)
    in_f = feat.shape[1]
    npos = _ceil_to(nloc, 128)
    npad = _ceil_to(n + 1, 1024)
    featT_g = np.zeros((in_f, npad), bf16)
    featT_g[:, :n] = (feat * cj[:, None]).T.astype(bf16)
    cj_pad = np.zeros(npad, np.float32)
    cj_pad[:n] = cj
    cj_sb = np.ascontiguousarray(cj_pad.reshape(-1, 128).T)
    w = np.ascontiguousarray(np.asarray(weight, np.float32).astype(bf16))
    wk = np.ascontiguousarray(np.asarray(weight_k, np.float32))
    in_maps = []
    for c, (perm, idx_arr) in enumerate(zip(perms, idx_arrs)):
        gids = c * nloc + perm
        fl = np.zeros((in_f, npos), np.float32)
        fl[:, :nloc] = feat[gids].T
        ci_pad = np.zeros(npos, np.float32)
        ci_pad[:nloc] = ci[gids]
        ci_sb = np.ascontiguousarray(ci_pad.reshape(-1, 128).T)
        in_maps.append({
            "featT_g": featT_g, "featT_l": fl, "w": w, "wk": wk,
            "cj_sb": cj_sb, "ci_sb": ci_sb, "idxs": idx_arr,
        })
    return in_maps


def decode_outputs(results, perms, n, nloc, out_f):
    npos = _ceil_to(nloc, 128)
    full = np.zeros((n, out_f), np.float32)
    for c, perm in enumerate(perms):
        ob = np.asarray(results[c]["out"])  # [128, npos]
        dec = ob.reshape(128, npos // 128, out_f).transpose(1, 0, 2)
        dec = dec.reshape(npos, out_f)
        full[c * nloc + perm] = dec[:nloc]
    return full


_CACHE = {}

CHUNK_BLOCKS = 8


def run(feat, ci, cj, weight, weight_k, src, dst, *, n=N, ncores=NCORES,
        in_f=IN_F, out_f=OUT_F, heads=HEADS, d_k=D_K, tau=TAU,
        chunk_blocks=CHUNK_BLOCKS, trace=False, tmpdir=None):
    from concourse.bass_utils import run_bass_kernel_spmd
    nloc = n // ncores
    perms, idx_arrs, segments, tb = plan(src, dst, n, nloc, ncores,
                                         chunk_blocks)
    seg_key = (n, ncores, tb, tuple(tuple(s) for ss in segments for s in ss),
               tuple(len(ss) for ss in segments))
    if seg_key in _CACHE:
        nc = _CACHE[seg_key]
    else:
        nc = build(n, in_f, out_f, heads, d_k, tau, nloc, tb, segments,
                   chunk_blocks)
        _CACHE[seg_key] = nc
    in_maps = make_inputs(feat, ci, cj, weight, weight_k, perms, idx_arrs,
                          n, nloc)
    res = run_bass_kernel_spmd(nc, in_maps, core_ids=list(range(ncores)),
                               trace=trace, tmpdir=tmpdir)
    out = decode_outputs(res.results, perms, n, nloc, out_f)
    return out, res


def kernel(feat, ci, cj, weight, weight_k, src, dst):
    out, _ = run(feat, ci, cj, weight, weight_k, src, dst)
    return out

